# revision 7
# baseline (speedup 1.0000x reference)
"""ASTRA contrastive loss on 8 Trainium2 NeuronCores (Bass/Tile).

Pure data parallel: batch B=1024 is sharded 128 samples per core. Each core
computes, for its 128 samples x 64 agents:
    dot[b,n]  = sum_d mut*heal   (DVE tensor_tensor mult + segmented reduce)
    ssm[b,n]  = sum_d mut*mut    (ACT Square with fused accum_out, per slab)
    ssh[b,n]  = sum_d heal*heal  (split between ACT-accum and ACT-square +
                                  DVE-reduce to balance the two engines)
then a tiny [128,64] epilogue produces per-sample (contrib, valid) pairs which
the host sums across cores and divides (the "all-reduce" of the hint — the
reduced payload is 2 floats/sample, so the host-side gather is the cheap form).

Layout: partition = sample (128 lanes = 128 samples per core), free = (agent,
dim). Input DMAs move 1 MiB per transfer (8 agents x 256 dims x 4B x 128
partitions, 8 KiB contiguous per partition) which is in the >=75%-of-peak DMA
regime. Engine budget per core: DVE ~46us, ACT ~45us, both at the ~47us HBM
floor for the 16.8 MiB of input — the kernel is DMA-bound at the ridge.

Note: tensor_tensor_reduce (custom ANT-DVE ucode) is NOT used — it crashes
the NRT runtime on this deployment (verified by bisection).
"""

import sys

import numpy as np

_REPO = "/opt/trn_rl_repo"
if _REPO not in sys.path:
    sys.path.insert(0, _REPO)

B, N, D = 1024, 64, 256
NCORES = 8
BP = B // NCORES          # samples per core (one SBUF partition each)
GROUPS = 8                # DMA chunks per tensor per core
GA = N // GROUPS          # agents per chunk -> 1 MiB per DMA
ACT_SSH_GROUPS = 0        # groups whose ssh reduce runs fully on ACT
MULT_ENGINE = "gpsimd"    # "dve" | "gpsimd": engine for the mut*heal product
SSM_MODE = "accum"        # "accum" (ACT fused) | "bigsq" (ACT square + DVE reduce)
SQ_FIRST = True           # emit the big ssh square before the ssm accum calls
MARGIN = 1.0
ALPHA = 0.7
EPS = 1e-8

_NC_CACHE = {}


def _build_nc(reps=1):
    """Build the single-core Bass/Tile program (SPMD across 8 cores).

    reps>1 wraps the whole body in a Tile For_i loop — used only by the
    benchmark harness to measure steady-state per-iteration HW time.
    """
    from contextlib import ExitStack

    import concourse.bacc as bacc
    import concourse.tile as tile
    from concourse import mybir

    f32 = mybir.dt.float32
    Alu = mybir.AluOpType
    Act = mybir.ActivationFunctionType

    nc = bacc.Bacc(None, target_bir_lowering=False, debug=False, num_devices=NCORES)
    mut_d = nc.declare_dram_parameter("emb_mut", [BP, N, D], f32, isOutput=False)
    heal_d = nc.declare_dram_parameter("emb_heal", [BP, N, D], f32, isOutput=False)
    idx_d = nc.declare_dram_parameter("idx_f", [BP, 1], f32, isOutput=False)
    mask_d = nc.declare_dram_parameter("mask_f", [BP, N], f32, isOutput=False)
    iota_d = nc.declare_dram_parameter("iota_f", [BP, N], f32, isOutput=False)
    out_d = nc.declare_dram_parameter("out", [BP, 2], f32, isOutput=True)

    def emit_body(tc, ctx, pools):
        (mut_pool, heal_pool, scr_pool, act_pool, st_pool, ep_pool) = pools

        dot = st_pool.tile([BP, N], f32, tag="dot")
        ssm = st_pool.tile([BP, N], f32, tag="ssm")
        ssh = st_pool.tile([BP, N], f32, tag="ssh")

        # ---- main reduction over groups of GA agents ----
        for g in range(GROUPS):
            gsl = slice(g * GA, (g + 1) * GA)
            mt = mut_pool.tile([BP, GA, D], f32, tag="mt")
            nc.sync.dma_start(out=mt[:, :, :], in_=mut_d[:, gsl, :])
            ht = heal_pool.tile([BP, GA, D], f32, tag="ht")
            nc.sync.dma_start(out=ht[:, :, :], in_=heal_d[:, gsl, :])

            # dot: elementwise mult + DVE segmented reduce
            prod = scr_pool.tile([BP, GA, D], f32, tag="prod")
            if MULT_ENGINE == "gpsimd":
                nc.gpsimd.tensor_tensor(out=prod[:, :, :], in0=mt[:, :, :],
                                        in1=ht[:, :, :], op=Alu.mult)
            else:
                nc.vector.tensor_tensor(out=prod[:, :, :], in0=mt[:, :, :],
                                        in1=ht[:, :, :], op=Alu.mult)
            nc.vector.tensor_reduce(out=dot[:, gsl], in_=prod[:, :, :],
                                    axis=mybir.AxisListType.X, op=Alu.add)

            def emit_ssh():
                # ssh: ACT-accum for the first ACT_SSH_GROUPS groups, else
                # ACT big square + DVE segmented reduce
                if g < ACT_SSH_GROUPS:
                    for j in range(GA):
                        n = g * GA + j
                        ascr2 = act_pool.tile([BP, D], f32, tag="ascr")
                        nc.scalar.activation(out=ascr2[:, :], in_=ht[:, j, :],
                                             func=Act.Square,
                                             accum_out=ssh[:, n:n + 1])
                else:
                    sqh = scr_pool.tile([BP, GA, D], f32, tag="prod")
                    nc.scalar.activation(out=sqh[:, :, :], in_=ht[:, :, :],
                                         func=Act.Square)
                    nc.vector.tensor_reduce(out=ssh[:, gsl], in_=sqh[:, :, :],
                                            axis=mybir.AxisListType.X, op=Alu.add)

            def emit_ssm():
                # ssm: ACT Square with fused accumulate, or big square + reduce
                if SSM_MODE == "accum":
                    for j in range(GA):
                        n = g * GA + j
                        ascr = act_pool.tile([BP, D], f32, tag="ascr")
                        nc.scalar.activation(out=ascr[:, :], in_=mt[:, j, :],
                                             func=Act.Square,
                                             accum_out=ssm[:, n:n + 1])
                else:
                    sqm = scr_pool.tile([BP, GA, D], f32, tag="prod")
                    nc.scalar.activation(out=sqm[:, :, :], in_=mt[:, :, :],
                                         func=Act.Square)
                    nc.vector.tensor_reduce(out=ssm[:, gsl], in_=sqm[:, :, :],
                                            axis=mybir.AxisListType.X, op=Alu.add)

            if SQ_FIRST:
                emit_ssh()
                emit_ssm()
            else:
                emit_ssm()
                emit_ssh()

        # ---- tiny per-sample epilogue, all [128, 64] / [128, 1] ----
        idx_t = ep_pool.tile([BP, 1], f32, tag="idx")
        nc.sync.dma_start(out=idx_t[:, :], in_=idx_d[:, :])
        mask_t = ep_pool.tile([BP, N], f32, tag="mask")
        nc.sync.dma_start(out=mask_t[:, :], in_=mask_d[:, :])
        iota_t = ep_pool.tile([BP, N], f32, tag="iota")
        nc.sync.dma_start(out=iota_t[:, :], in_=iota_d[:, :])

        # cos = dot / sqrt(ssm*ssh), with one Newton step on the sqrt
        # (ACT Sqrt has a loose ULP budget; Newton: den' = 0.5*(den+den2/den)).
        den2 = ep_pool.tile([BP, N], f32, tag="den2")
        nc.vector.tensor_tensor(out=den2[:, :], in0=ssm[:, :], in1=ssh[:, :],
                                op=Alu.mult)
        den = ep_pool.tile([BP, N], f32, tag="den")
        nc.scalar.activation(out=den[:, :], in_=den2[:, :], func=Act.Sqrt)
        rden = ep_pool.tile([BP, N], f32, tag="rden")
        nc.vector.reciprocal(out=rden[:, :], in_=den[:, :])
        t0 = ep_pool.tile([BP, N], f32, tag="t0")
        nc.vector.tensor_tensor(out=t0[:, :], in0=den2[:, :], in1=rden[:, :],
                                op=Alu.mult)
        nc.vector.tensor_tensor(out=den[:, :], in0=den[:, :], in1=t0[:, :],
                                op=Alu.add)
        nc.vector.tensor_scalar(out=den[:, :], in0=den[:, :], scalar1=0.5,
                                scalar2=EPS * EPS, op0=Alu.mult, op1=Alu.max)
        inv = ep_pool.tile([BP, N], f32, tag="inv")
        nc.vector.reciprocal(out=inv[:, :], in_=den[:, :])
        cos = ep_pool.tile([BP, N], f32, tag="cos")
        nc.vector.tensor_tensor(out=cos[:, :], in0=dot[:, :], in1=inv[:, :],
                                op=Alu.mult)

        # validity and clipped index
        v0 = ep_pool.tile([BP, 1], f32, tag="v0")
        nc.vector.tensor_scalar(out=v0[:, :], in0=idx_t[:, :], scalar1=0.0,
                                scalar2=None, op0=Alu.is_ge)
        v1 = ep_pool.tile([BP, 1], f32, tag="v1")
        nc.vector.tensor_scalar(out=v1[:, :], in0=idx_t[:, :], scalar1=float(N),
                                scalar2=None, op0=Alu.is_lt)
        valid = ep_pool.tile([BP, 1], f32, tag="valid")
        nc.vector.tensor_tensor(out=valid[:, :], in0=v0[:, :], in1=v1[:, :],
                                op=Alu.mult)
        idx_c = ep_pool.tile([BP, 1], f32, tag="idxc")
        nc.vector.tensor_scalar(out=idx_c[:, :], in0=idx_t[:, :], scalar1=0.0,
                                scalar2=float(N - 1), op0=Alu.max, op1=Alu.min)

        # one-hot of target agent; cos at target
        onehot = ep_pool.tile([BP, N], f32, tag="onehot")
        nc.vector.tensor_scalar(out=onehot[:, :], in0=iota_t[:, :],
                                scalar1=idx_c[:, 0:1], scalar2=None,
                                op0=Alu.is_equal)
        ct_prod = ep_pool.tile([BP, N], f32, tag="ctprod")
        nc.vector.tensor_tensor(out=ct_prod[:, :], in0=cos[:, :],
                                in1=onehot[:, :], op=Alu.mult)
        cos_t = ep_pool.tile([BP, 1], f32, tag="cost")
        nc.vector.tensor_reduce(out=cos_t[:, :], in_=ct_prod[:, :],
                                axis=mybir.AxisListType.X, op=Alu.add)
        # loss_target = relu(cos_t + margin)
        loss_t = ep_pool.tile([BP, 1], f32, tag="losst")
        nc.vector.tensor_scalar(out=loss_t[:, :], in0=cos_t[:, :],
                                scalar1=MARGIN, scalar2=0.0,
                                op0=Alu.add, op1=Alu.max)

        # normal mask, count, sum of (1-cos) over normal agents
        notT = ep_pool.tile([BP, N], f32, tag="notT")
        nc.vector.tensor_scalar(out=notT[:, :], in0=onehot[:, :], scalar1=-1.0,
                                scalar2=1.0, op0=Alu.mult, op1=Alu.add)
        nmask = ep_pool.tile([BP, N], f32, tag="nmask")
        nc.vector.tensor_tensor(out=nmask[:, :], in0=mask_t[:, :],
                                in1=notT[:, :], op=Alu.mult)
        cnt = ep_pool.tile([BP, 1], f32, tag="cnt")
        nc.vector.tensor_reduce(out=cnt[:, :], in_=nmask[:, :],
                                axis=mybir.AxisListType.X, op=Alu.add)
        mc_prod = ep_pool.tile([BP, N], f32, tag="mcprod")
        nc.vector.tensor_tensor(out=mc_prod[:, :], in0=nmask[:, :],
                                in1=cos[:, :], op=Alu.mult)
        mc = ep_pool.tile([BP, 1], f32, tag="mc")
        nc.vector.tensor_reduce(out=mc[:, :], in_=mc_prod[:, :],
                                axis=mybir.AxisListType.X, op=Alu.add)
        so = ep_pool.tile([BP, 1], f32, tag="so")
        nc.vector.tensor_tensor(out=so[:, :], in0=cnt[:, :], in1=mc[:, :],
                                op=Alu.subtract)

        # loss_others = (cnt>0) ? so / max(cnt,1) : 0
        cnt1 = ep_pool.tile([BP, 1], f32, tag="cnt1")
        nc.vector.tensor_scalar(out=cnt1[:, :], in0=cnt[:, :], scalar1=1.0,
                                scalar2=None, op0=Alu.max)
        icnt = ep_pool.tile([BP, 1], f32, tag="icnt")
        nc.vector.reciprocal(out=icnt[:, :], in_=cnt1[:, :])
        gpos = ep_pool.tile([BP, 1], f32, tag="gpos")
        nc.vector.tensor_scalar(out=gpos[:, :], in0=cnt[:, :], scalar1=0.0,
                                scalar2=None, op0=Alu.is_gt)
        lo = ep_pool.tile([BP, 1], f32, tag="lo")
        nc.vector.tensor_tensor(out=lo[:, :], in0=so[:, :], in1=icnt[:, :],
                                op=Alu.mult)
        nc.vector.tensor_tensor(out=lo[:, :], in0=lo[:, :], in1=gpos[:, :],
                                op=Alu.mult)

        # per-sample loss, gated by validity
        pa = ep_pool.tile([BP, 1], f32, tag="pa")
        nc.vector.tensor_scalar(out=pa[:, :], in0=loss_t[:, :], scalar1=ALPHA,
                                scalar2=None, op0=Alu.mult)
        pb = ep_pool.tile([BP, 1], f32, tag="pb")
        nc.vector.tensor_scalar(out=pb[:, :], in0=lo[:, :],
                                scalar1=1.0 - ALPHA, scalar2=None, op0=Alu.mult)
        per = ep_pool.tile([BP, 1], f32, tag="per")
        nc.vector.tensor_tensor(out=per[:, :], in0=pa[:, :], in1=pb[:, :],
                                op=Alu.add)
        contrib = ep_pool.tile([BP, 1], f32, tag="contrib")
        nc.vector.tensor_tensor(out=contrib[:, :], in0=per[:, :],
                                in1=valid[:, :], op=Alu.mult)

        out_sb = ep_pool.tile([BP, 2], f32, tag="outsb")
        nc.vector.tensor_copy(out_sb[:, 0:1], contrib[:, :])
        nc.vector.tensor_copy(out_sb[:, 1:2], valid[:, :])
        nc.sync.dma_start(out=out_d[:, :], in_=out_sb[:, :])

    with tile.TileContext(nc) as tc, ExitStack() as ctx:
        pools = (
            ctx.enter_context(tc.tile_pool(name="mut", bufs=GROUPS)),
            ctx.enter_context(tc.tile_pool(name="heal", bufs=GROUPS)),
            ctx.enter_context(tc.tile_pool(name="scr", bufs=3)),
            ctx.enter_context(tc.tile_pool(name="ascr", bufs=3)),
            ctx.enter_context(tc.tile_pool(name="stats", bufs=2)),
            ctx.enter_context(tc.tile_pool(name="epi", bufs=2)),
        )
        if reps == 1:
            emit_body(tc, ctx, pools)
        else:
            with tc.For_i(0, reps, 1):
                emit_body(tc, ctx, pools)

    nc.compile()
    return nc


def _get_nc(reps=1):
    key = ("nc", reps)
    if key not in _NC_CACHE:
        _NC_CACHE[key] = _build_nc(reps)
    return _NC_CACHE[key]


def _make_in_maps(inputs):
    mut = np.ascontiguousarray(inputs["emb_mut"], dtype=np.float32)
    heal = np.ascontiguousarray(inputs["emb_heal"], dtype=np.float32)
    idx_f = np.asarray(inputs["mistake_agent_idx"]).astype(np.float32).reshape(B, 1)
    mask_f = np.asarray(inputs["agent_mask"]).astype(np.float32).reshape(B, N)
    iota_f = np.ascontiguousarray(
        np.broadcast_to(np.arange(N, dtype=np.float32), (BP, N))
    )
    in_maps = []
    for c in range(NCORES):
        sl = slice(c * BP, (c + 1) * BP)
        in_maps.append({
            "emb_mut": mut[sl],
            "emb_heal": heal[sl],
            "idx_f": np.ascontiguousarray(idx_f[sl]),
            "mask_f": np.ascontiguousarray(mask_f[sl]),
            "iota_f": iota_f,
        })
    return in_maps


def run_spmd(inputs, trace=False, reps=1):
    """Run on all 8 cores; returns (final_scalar, BassKernelResults)."""
    from concourse.bass_utils import run_bass_kernel_spmd

    nc = _get_nc(reps)
    in_maps = _make_in_maps(inputs)
    res = run_bass_kernel_spmd(nc, in_maps, list(range(NCORES)), trace=trace)
    outs = np.stack([r["out"] for r in res.results])  # [8, 128, 2]
    total = outs[..., 0].sum(dtype=np.float64)
    count = outs[..., 1].sum(dtype=np.float64)
    val = np.float32(total / count) if count > 0 else np.float32(0.0)
    return val, res


def kernel(**inputs) -> np.ndarray:
    val, _ = run_spmd(inputs, trace=False)
    return val


# revision 11
# speedup vs baseline: 1.4894x; 1.4894x over previous
"""ASTRA contrastive loss on 8 Trainium2 NeuronCores (Bass/Tile).

Pure data parallel: batch B=1024 is sharded 128 samples per core. Each core
computes, for its 128 samples x 64 agents:
    dot[b,n]  = sum_d mut*heal   (DVE tensor_tensor mult + segmented reduce)
    ssm[b,n]  = sum_d mut*mut    (ACT Square with fused accum_out, per slab)
    ssh[b,n]  = sum_d heal*heal  (split between ACT-accum and ACT-square +
                                  DVE-reduce to balance the two engines)
then a tiny [128,64] epilogue produces per-sample (contrib, valid) pairs which
the host sums across cores and divides (the "all-reduce" of the hint — the
reduced payload is 2 floats/sample, so the host-side gather is the cheap form).

Layout: partition = sample (128 lanes = 128 samples per core), free = (agent,
dim). Input DMAs move 1 MiB per transfer (8 agents x 256 dims x 4B x 128
partitions, 8 KiB contiguous per partition) which is in the >=75%-of-peak DMA
regime. Engine budget per core: DVE ~46us, ACT ~45us, both at the ~47us HBM
floor for the 16.8 MiB of input — the kernel is DMA-bound at the ridge.

Note: tensor_tensor_reduce (custom ANT-DVE ucode) is NOT used — it crashes
the NRT runtime on this deployment (verified by bisection).
"""

import sys

import numpy as np

_REPO = "/opt/trn_rl_repo"
if _REPO not in sys.path:
    sys.path.insert(0, _REPO)

B, N, D = 1024, 64, 256
NCORES = 8
BP = B // NCORES          # samples per core (one SBUF partition each)
GROUPS = 8                # DMA chunks per tensor per core
GA = N // GROUPS          # agents per chunk -> 1 MiB per DMA
ACT_SSH_GROUPS = 0        # groups whose ssh reduce runs fully on ACT
MULT_ENGINE = "gpsimd"    # "dve" | "gpsimd": engine for the mut*heal product
SSM_MODE = "accum"        # "accum" (ACT fused) | "bigsq" (ACT square + DVE reduce)
SQ_FIRST = True           # emit the big ssh square before the ssm accum calls
KPARTS = "full"           # "full" | "dma" | "dot": bench bisection (dma = loads only)
MARGIN = 1.0
ALPHA = 0.7
EPS = 1e-8

_NC_CACHE = {}


def _build_nc(reps=1):
    """Build the single-core Bass/Tile program (SPMD across 8 cores).

    reps>1 wraps the whole body in a Tile For_i loop — used only by the
    benchmark harness to measure steady-state per-iteration HW time.
    """
    from contextlib import ExitStack

    import concourse.bacc as bacc
    import concourse.tile as tile
    from concourse import mybir

    f32 = mybir.dt.float32
    Alu = mybir.AluOpType
    Act = mybir.ActivationFunctionType

    nc = bacc.Bacc(None, target_bir_lowering=False, debug=False, num_devices=NCORES)
    mut_d = nc.declare_dram_parameter("emb_mut", [BP, N, D], f32, isOutput=False)
    heal_d = nc.declare_dram_parameter("emb_heal", [BP, N, D], f32, isOutput=False)
    idx_d = nc.declare_dram_parameter("idx_f", [BP, 1], f32, isOutput=False)
    mask_d = nc.declare_dram_parameter("mask_f", [BP, N], f32, isOutput=False)
    iota_d = nc.declare_dram_parameter("iota_f", [BP, N], f32, isOutput=False)
    out_d = nc.declare_dram_parameter("out", [BP, 2], f32, isOutput=True)

    def emit_body(tc, ctx, pools):
        (mut_pool, heal_pool, scr_pool, act_pool, st_pool, ep_pool) = pools

        dot = st_pool.tile([BP, N], f32, tag="dot")
        ssm = st_pool.tile([BP, N], f32, tag="ssm")
        ssh = st_pool.tile([BP, N], f32, tag="ssh")

        # ---- main reduction over groups of GA agents ----
        for g in range(GROUPS):
            gsl = slice(g * GA, (g + 1) * GA)
            mt = mut_pool.tile([BP, GA, D], f32, tag="mt")
            nc.sync.dma_start(out=mt[:, :, :], in_=mut_d[:, gsl, :])
            ht = heal_pool.tile([BP, GA, D], f32, tag="ht")
            nc.sync.dma_start(out=ht[:, :, :], in_=heal_d[:, gsl, :])

            if KPARTS == "dma":
                continue
            # dot: elementwise mult + DVE segmented reduce
            prod = scr_pool.tile([BP, GA, D], f32, tag="prod")
            if MULT_ENGINE == "gpsimd":
                nc.gpsimd.tensor_tensor(out=prod[:, :, :], in0=mt[:, :, :],
                                        in1=ht[:, :, :], op=Alu.mult)
            else:
                nc.vector.tensor_tensor(out=prod[:, :, :], in0=mt[:, :, :],
                                        in1=ht[:, :, :], op=Alu.mult)
            nc.vector.tensor_reduce(out=dot[:, gsl], in_=prod[:, :, :],
                                    axis=mybir.AxisListType.X, op=Alu.add)
            if KPARTS == "dot":
                continue

            def emit_ssh():
                # ssh: ACT-accum for the first ACT_SSH_GROUPS groups, else
                # ACT big square + DVE segmented reduce
                if g < ACT_SSH_GROUPS:
                    for j in range(GA):
                        n = g * GA + j
                        ascr2 = act_pool.tile([BP, D], f32, tag="ascr")
                        nc.scalar.activation(out=ascr2[:, :], in_=ht[:, j, :],
                                             func=Act.Square,
                                             accum_out=ssh[:, n:n + 1])
                else:
                    sqh = scr_pool.tile([BP, GA, D], f32, tag="prod")
                    nc.scalar.activation(out=sqh[:, :, :], in_=ht[:, :, :],
                                         func=Act.Square)
                    nc.vector.tensor_reduce(out=ssh[:, gsl], in_=sqh[:, :, :],
                                            axis=mybir.AxisListType.X, op=Alu.add)

            def emit_ssm():
                # ssm: ACT Square with fused accumulate, or big square + reduce
                if SSM_MODE == "accum":
                    for j in range(GA):
                        n = g * GA + j
                        ascr = act_pool.tile([BP, D], f32, tag="ascr")
                        nc.scalar.activation(out=ascr[:, :], in_=mt[:, j, :],
                                             func=Act.Square,
                                             accum_out=ssm[:, n:n + 1])
                else:
                    sqm = scr_pool.tile([BP, GA, D], f32, tag="prod")
                    nc.scalar.activation(out=sqm[:, :, :], in_=mt[:, :, :],
                                         func=Act.Square)
                    nc.vector.tensor_reduce(out=ssm[:, gsl], in_=sqm[:, :, :],
                                            axis=mybir.AxisListType.X, op=Alu.add)

            if SQ_FIRST:
                emit_ssh()
                emit_ssm()
            else:
                emit_ssm()
                emit_ssh()

        if KPARTS != "full":
            out_sb0 = ep_pool.tile([BP, 2], f32, tag="outsb")
            if KPARTS == "dma":
                nc.vector.memset(out_sb0[:, :], 0.0)
            else:
                nc.vector.tensor_copy(out_sb0[:, :], dot[:, 0:2])
            nc.sync.dma_start(out=out_d[:, :], in_=out_sb0[:, :])
            return

        # ---- tiny per-sample epilogue, all [128, 64] / [128, 1] ----
        idx_t = ep_pool.tile([BP, 1], f32, tag="idx")
        nc.sync.dma_start(out=idx_t[:, :], in_=idx_d[:, :])
        mask_t = ep_pool.tile([BP, N], f32, tag="mask")
        nc.sync.dma_start(out=mask_t[:, :], in_=mask_d[:, :])
        iota_t = ep_pool.tile([BP, N], f32, tag="iota")
        nc.sync.dma_start(out=iota_t[:, :], in_=iota_d[:, :])

        # cos = dot / sqrt(ssm*ssh), with one Newton step on the sqrt
        # (ACT Sqrt has a loose ULP budget; Newton: den' = 0.5*(den+den2/den)).
        den2 = ep_pool.tile([BP, N], f32, tag="den2")
        nc.vector.tensor_tensor(out=den2[:, :], in0=ssm[:, :], in1=ssh[:, :],
                                op=Alu.mult)
        den = ep_pool.tile([BP, N], f32, tag="den")
        nc.scalar.activation(out=den[:, :], in_=den2[:, :], func=Act.Sqrt)
        rden = ep_pool.tile([BP, N], f32, tag="rden")
        nc.vector.reciprocal(out=rden[:, :], in_=den[:, :])
        t0 = ep_pool.tile([BP, N], f32, tag="t0")
        nc.vector.tensor_tensor(out=t0[:, :], in0=den2[:, :], in1=rden[:, :],
                                op=Alu.mult)
        nc.vector.tensor_tensor(out=den[:, :], in0=den[:, :], in1=t0[:, :],
                                op=Alu.add)
        nc.vector.tensor_scalar(out=den[:, :], in0=den[:, :], scalar1=0.5,
                                scalar2=EPS * EPS, op0=Alu.mult, op1=Alu.max)
        inv = ep_pool.tile([BP, N], f32, tag="inv")
        nc.vector.reciprocal(out=inv[:, :], in_=den[:, :])
        cos = ep_pool.tile([BP, N], f32, tag="cos")
        nc.vector.tensor_tensor(out=cos[:, :], in0=dot[:, :], in1=inv[:, :],
                                op=Alu.mult)

        # validity and clipped index
        v0 = ep_pool.tile([BP, 1], f32, tag="v0")
        nc.vector.tensor_scalar(out=v0[:, :], in0=idx_t[:, :], scalar1=0.0,
                                scalar2=None, op0=Alu.is_ge)
        v1 = ep_pool.tile([BP, 1], f32, tag="v1")
        nc.vector.tensor_scalar(out=v1[:, :], in0=idx_t[:, :], scalar1=float(N),
                                scalar2=None, op0=Alu.is_lt)
        valid = ep_pool.tile([BP, 1], f32, tag="valid")
        nc.vector.tensor_tensor(out=valid[:, :], in0=v0[:, :], in1=v1[:, :],
                                op=Alu.mult)
        idx_c = ep_pool.tile([BP, 1], f32, tag="idxc")
        nc.vector.tensor_scalar(out=idx_c[:, :], in0=idx_t[:, :], scalar1=0.0,
                                scalar2=float(N - 1), op0=Alu.max, op1=Alu.min)

        # one-hot of target agent; cos at target
        onehot = ep_pool.tile([BP, N], f32, tag="onehot")
        nc.vector.tensor_scalar(out=onehot[:, :], in0=iota_t[:, :],
                                scalar1=idx_c[:, 0:1], scalar2=None,
                                op0=Alu.is_equal)
        ct_prod = ep_pool.tile([BP, N], f32, tag="ctprod")
        nc.vector.tensor_tensor(out=ct_prod[:, :], in0=cos[:, :],
                                in1=onehot[:, :], op=Alu.mult)
        cos_t = ep_pool.tile([BP, 1], f32, tag="cost")
        nc.vector.tensor_reduce(out=cos_t[:, :], in_=ct_prod[:, :],
                                axis=mybir.AxisListType.X, op=Alu.add)
        # loss_target = relu(cos_t + margin)
        loss_t = ep_pool.tile([BP, 1], f32, tag="losst")
        nc.vector.tensor_scalar(out=loss_t[:, :], in0=cos_t[:, :],
                                scalar1=MARGIN, scalar2=0.0,
                                op0=Alu.add, op1=Alu.max)

        # normal mask, count, sum of (1-cos) over normal agents
        notT = ep_pool.tile([BP, N], f32, tag="notT")
        nc.vector.tensor_scalar(out=notT[:, :], in0=onehot[:, :], scalar1=-1.0,
                                scalar2=1.0, op0=Alu.mult, op1=Alu.add)
        nmask = ep_pool.tile([BP, N], f32, tag="nmask")
        nc.vector.tensor_tensor(out=nmask[:, :], in0=mask_t[:, :],
                                in1=notT[:, :], op=Alu.mult)
        cnt = ep_pool.tile([BP, 1], f32, tag="cnt")
        nc.vector.tensor_reduce(out=cnt[:, :], in_=nmask[:, :],
                                axis=mybir.AxisListType.X, op=Alu.add)
        mc_prod = ep_pool.tile([BP, N], f32, tag="mcprod")
        nc.vector.tensor_tensor(out=mc_prod[:, :], in0=nmask[:, :],
                                in1=cos[:, :], op=Alu.mult)
        mc = ep_pool.tile([BP, 1], f32, tag="mc")
        nc.vector.tensor_reduce(out=mc[:, :], in_=mc_prod[:, :],
                                axis=mybir.AxisListType.X, op=Alu.add)
        so = ep_pool.tile([BP, 1], f32, tag="so")
        nc.vector.tensor_tensor(out=so[:, :], in0=cnt[:, :], in1=mc[:, :],
                                op=Alu.subtract)

        # loss_others = (cnt>0) ? so / max(cnt,1) : 0
        cnt1 = ep_pool.tile([BP, 1], f32, tag="cnt1")
        nc.vector.tensor_scalar(out=cnt1[:, :], in0=cnt[:, :], scalar1=1.0,
                                scalar2=None, op0=Alu.max)
        icnt = ep_pool.tile([BP, 1], f32, tag="icnt")
        nc.vector.reciprocal(out=icnt[:, :], in_=cnt1[:, :])
        gpos = ep_pool.tile([BP, 1], f32, tag="gpos")
        nc.vector.tensor_scalar(out=gpos[:, :], in0=cnt[:, :], scalar1=0.0,
                                scalar2=None, op0=Alu.is_gt)
        lo = ep_pool.tile([BP, 1], f32, tag="lo")
        nc.vector.tensor_tensor(out=lo[:, :], in0=so[:, :], in1=icnt[:, :],
                                op=Alu.mult)
        nc.vector.tensor_tensor(out=lo[:, :], in0=lo[:, :], in1=gpos[:, :],
                                op=Alu.mult)

        # per-sample loss, gated by validity
        pa = ep_pool.tile([BP, 1], f32, tag="pa")
        nc.vector.tensor_scalar(out=pa[:, :], in0=loss_t[:, :], scalar1=ALPHA,
                                scalar2=None, op0=Alu.mult)
        pb = ep_pool.tile([BP, 1], f32, tag="pb")
        nc.vector.tensor_scalar(out=pb[:, :], in0=lo[:, :],
                                scalar1=1.0 - ALPHA, scalar2=None, op0=Alu.mult)
        per = ep_pool.tile([BP, 1], f32, tag="per")
        nc.vector.tensor_tensor(out=per[:, :], in0=pa[:, :], in1=pb[:, :],
                                op=Alu.add)
        contrib = ep_pool.tile([BP, 1], f32, tag="contrib")
        nc.vector.tensor_tensor(out=contrib[:, :], in0=per[:, :],
                                in1=valid[:, :], op=Alu.mult)

        out_sb = ep_pool.tile([BP, 2], f32, tag="outsb")
        nc.vector.tensor_copy(out_sb[:, 0:1], contrib[:, :])
        nc.vector.tensor_copy(out_sb[:, 1:2], valid[:, :])
        nc.sync.dma_start(out=out_d[:, :], in_=out_sb[:, :])

    with tile.TileContext(nc) as tc, ExitStack() as ctx:
        pools = (
            ctx.enter_context(tc.tile_pool(name="mut", bufs=GROUPS)),
            ctx.enter_context(tc.tile_pool(name="heal", bufs=GROUPS)),
            ctx.enter_context(tc.tile_pool(name="scr", bufs=3)),
            ctx.enter_context(tc.tile_pool(name="ascr", bufs=3)),
            ctx.enter_context(tc.tile_pool(name="stats", bufs=2)),
            ctx.enter_context(tc.tile_pool(name="epi", bufs=2)),
        )
        if reps == 1:
            emit_body(tc, ctx, pools)
        else:
            with tc.For_i(0, reps, 1):
                emit_body(tc, ctx, pools)

    nc.compile()
    return nc


def _get_nc(reps=1):
    key = ("nc", reps)
    if key not in _NC_CACHE:
        _NC_CACHE[key] = _build_nc(reps)
    return _NC_CACHE[key]


def _make_in_maps(inputs):
    mut = np.ascontiguousarray(inputs["emb_mut"], dtype=np.float32)
    heal = np.ascontiguousarray(inputs["emb_heal"], dtype=np.float32)
    idx_f = np.asarray(inputs["mistake_agent_idx"]).astype(np.float32).reshape(B, 1)
    mask_f = np.asarray(inputs["agent_mask"]).astype(np.float32).reshape(B, N)
    iota_f = np.ascontiguousarray(
        np.broadcast_to(np.arange(N, dtype=np.float32), (BP, N))
    )
    in_maps = []
    for c in range(NCORES):
        sl = slice(c * BP, (c + 1) * BP)
        in_maps.append({
            "emb_mut": mut[sl],
            "emb_heal": heal[sl],
            "idx_f": np.ascontiguousarray(idx_f[sl]),
            "mask_f": np.ascontiguousarray(mask_f[sl]),
            "iota_f": iota_f,
        })
    return in_maps


def run_spmd(inputs, trace=False, reps=1):
    """Run on all 8 cores; returns (final_scalar, BassKernelResults)."""
    from concourse.bass_utils import run_bass_kernel_spmd

    nc = _get_nc(reps)
    in_maps = _make_in_maps(inputs)
    res = run_bass_kernel_spmd(nc, in_maps, list(range(NCORES)), trace=trace)
    outs = np.stack([r["out"] for r in res.results])  # [8, 128, 2]
    total = outs[..., 0].sum(dtype=np.float64)
    count = outs[..., 1].sum(dtype=np.float64)
    val = np.float32(total / count) if count > 0 else np.float32(0.0)
    return val, res


def kernel(**inputs) -> np.ndarray:
    val, _ = run_spmd(inputs, trace=False)
    return val
